# revision 2
# baseline (speedup 1.0000x reference)
"""Trainium2 Bass kernel for nn_O3TensorProductWeighted.

Computes, for each sample e:
    h  = relu(relu(weight @ W0 + b0) @ W1 + b1)           # [64]
    w  = h @ W2 + b2                                      # [36864] (never materialized)
    out0 = PW0*(einsum(Wa,s1)*s2 + I3*einsum(Wd,dot12))
    out1 = PW1*I3*(einsum(Wb,s1) x v2 + einsum(Wc,v1)*s2)
    out  = concat(out0, out1)/SQRT_K ; out[:128] += bias

Strategy: reassociate each einsum against the (k,u)-joint contraction of the
per-sample Khatri-Rao product h (x) x, so everything becomes dense matmuls
over shared W2 chunk weights, with the per-sample products built on-chip.
16 chunks of 4 k-values each; one PE broadcast matmul per chunk replicates
the 4 h2 rows 32x, ACT applies the relu (with a 2^2 scale folded in), and
two scalar_tensor_tensor DVE ops per chunk build all products at once.

The A (0e x 0e) and B (0e x 1o) paths run as fp8-e4m3 DoubleRow matmuls
(two 128-row tiles summed per instruction at 0.5 cycles/row): weights are
split into an fp8 main + same-scale fp8 residual (two DR instructions) so
only the fp8 product rounding contributes error. C/D paths stay bf16.
Residual DR work is deferred to the second half of the chunk loop so the
residual weight DMAs never gate the PE. Scale compensation (alpha*beta) is
folded into host-prescaled epilogue operands. Pure data parallel over 8
cores, transposed layout (features on partitions, samples on free dim).
"""

import dataclasses
import sys

sys.path.insert(0, "/opt/trn_rl_repo")

from contextlib import ExitStack

import ml_dtypes
import numpy as np

import concourse.bacc as bacc
import concourse.bass as bass
import concourse.tile as tile
from concourse import mybir
from concourse.bass_utils import run_bass_kernel_spmd

BF16 = mybir.dt.bfloat16
FP8 = mybir.dt.float8e4
F32 = mybir.dt.float32
BF16_NP = ml_dtypes.bfloat16
FP8_NP = ml_dtypes.float8_e4m3

N_CORES = 8
N = 4096
E = N // N_CORES  # 512 samples per core

MUL0, MUL1 = 128, 64
N1 = MUL0 * MUL0          # 16384
N2 = MUL0 * MUL1          # 8192
N3 = MUL1 * MUL1          # 4096
I3 = float(1.0 / np.sqrt(3.0))
# PW0/SQRT_K == 1.0 and PW1*I3/SQRT_K == 1.0 exactly; only I3 remains on D.

KPC = 4                   # k-values per chunk
G = 64 // KPC             # 16 chunks
ALPHA = 1024.0            # fp8 scale on A/B stationary weights
BETA = 4.0                # scale folded into the relu -> rides on products
AB_INV = 1.0 / (ALPHA * BETA)


def _build_nc():
    nc = bacc.Bacc(None)

    # per-core inputs, transposed [feature, E]
    wT_d = nc.declare_dram_parameter("wT", [16, E], BF16, isOutput=False)
    # fused x-side blocks, [128, 12*E]: 4 A/B (s1 32-slices, tiled 4x) then
    # 6 C (v1*s2 per comp, 32-slices tiled) then 2 D (dot12 32-slices tiled)
    fin0_d = nc.declare_dram_parameter("fin0", [128, 12 * E], BF16, isOutput=False)
    s1t_d = nc.declare_dram_parameter("s1t", [128, E], BF16, isOutput=False)
    vsall_d = nc.declare_dram_parameter("vsall", [64, 3 * E], BF16, isOutput=False)
    d2t_d = nc.declare_dram_parameter("d2t", [64, E], BF16, isOutput=False)
    s2b_d = nc.declare_dram_parameter("s2b", [128, E], F32, isOutput=False)
    v2b_d = [
        nc.declare_dram_parameter(f"v2b{i}", [64, E], F32, isOutput=False)
        for i in range(3)
    ]

    # replicated parameters
    w0_d = nc.declare_dram_parameter("w0", [16, 64], BF16, isOutput=False)
    b0c_d = nc.declare_dram_parameter("b0c", [64, 1], F32, isOutput=False)
    wg1_d = nc.declare_dram_parameter("wg1", [64, G * 128], BF16, isOutput=False)
    bg1_d = nc.declare_dram_parameter("bg1", [128, G], F32, isOutput=False)
    # A/B stationaries: [128, G, 2(jp), 2(tile), W] fp8 main + residual
    wa8m_d = nc.declare_dram_parameter("wa8m", [128, G * 512], FP8, isOutput=False)
    wa8r_d = nc.declare_dram_parameter("wa8r", [128, G * 512], FP8, isOutput=False)
    wb8m_d = nc.declare_dram_parameter("wb8m", [128, G * 256], FP8, isOutput=False)
    wb8r_d = nc.declare_dram_parameter("wb8r", [128, G * 256], FP8, isOutput=False)
    # C/D stationaries: [128, G, 2(j), W] bf16
    wc_d = nc.declare_dram_parameter("wc", [128, G * 128], BF16, isOutput=False)
    wd_d = nc.declare_dram_parameter("wd", [128, G * 256], BF16, isOutput=False)
    ba_d = nc.declare_dram_parameter("ba", [128, 128], BF16, isOutput=False)
    bb_d = nc.declare_dram_parameter("bb", [128, 64], BF16, isOutput=False)
    bc_d = nc.declare_dram_parameter("bc", [64, 64], BF16, isOutput=False)
    bd_d = nc.declare_dram_parameter("bd", [64, 128], BF16, isOutput=False)
    bcol_d = nc.declare_dram_parameter("bcol", [128, 1], F32, isOutput=False)
    ident_d = nc.declare_dram_parameter("ident", [128, 128], F32, isOutput=False)

    outp_d = nc.declare_dram_parameter("outp", [E, 320], F32, isOutput=True)

    with tile.TileContext(nc) as tc, ExitStack() as ctx:
        const = ctx.enter_context(tc.tile_pool(name="const", bufs=1))
        work = ctx.enter_context(tc.tile_pool(name="work", bufs=1))
        bct_pool = ctx.enter_context(tc.tile_pool(name="bct", bufs=3))
        # pt8 tiles must survive ~8 chunks (residual DRs are deferred)
        pt8_pool = ctx.enter_context(tc.tile_pool(name="pt8", bufs=18))
        ptc_pool = ctx.enter_context(tc.tile_pool(name="ptc", bufs=3))
        out_pool = ctx.enter_context(tc.tile_pool(name="outs", bufs=2))
        ps_acc = ctx.enter_context(tc.tile_pool(name="ps_acc", bufs=1, space="PSUM"))
        ps_rot = ctx.enter_context(tc.tile_pool(name="ps_rot", bufs=2, space="PSUM"))

        def load(dparam, engine):
            t = const.tile(dparam.shape, dparam.dtype, name=f"t_{dparam.name}")
            engine.dma_start(t[:], dparam[:])
            return t

        # small tensors via gpsimd SWDGE; big streams serialized on SP HWDGE
        wT_t = load(wT_d, nc.gpsimd)
        w0_t = load(w0_d, nc.gpsimd)
        b0c_t = load(b0c_d, nc.gpsimd)
        wg1_t = load(wg1_d, nc.gpsimd)
        bg1_t = load(bg1_d, nc.gpsimd)
        s1t_t = load(s1t_d, nc.gpsimd)
        vsall_t = load(vsall_d, nc.gpsimd)
        d2t_t = load(d2t_d, nc.gpsimd)
        ba_t = load(ba_d, nc.gpsimd)
        bb_t = load(bb_d, nc.gpsimd)
        bc_t = load(bc_d, nc.gpsimd)
        bd_t = load(bd_d, nc.gpsimd)
        bcol_t = load(bcol_d, nc.gpsimd)
        ident_t = load(ident_d, nc.gpsimd)

        fin0_t = load(fin0_d, nc.sync)
        wa8m_t = load(wa8m_d, nc.sync)
        wb8m_t = load(wb8m_d, nc.sync)
        wc_t = load(wc_d, nc.sync)
        wd_t = load(wd_d, nc.sync)
        wa8r_t = load(wa8r_d, nc.sync)
        wb8r_t = load(wb8r_d, nc.sync)
        s2b_t = load(s2b_d, nc.sync)
        v2b_t = [load(d, nc.sync) for d in v2b_d]

        f12 = fin0_t[:].rearrange("p (b e) -> p b e", b=12)
        wa8m4 = wa8m_t[:].rearrange("p (g j t w) -> p g j t w", g=G, j=2, t=2)
        wa8r4 = wa8r_t[:].rearrange("p (g j t w) -> p g j t w", g=G, j=2, t=2)
        wb8m4 = wb8m_t[:].rearrange("p (g j t w) -> p g j t w", g=G, j=2, t=2)
        wb8r4 = wb8r_t[:].rearrange("p (g j t w) -> p g j t w", g=G, j=2, t=2)
        wc3 = wc_t[:].rearrange("p (g j w) -> p g j w", g=G, j=2)
        wd3 = wd_t[:].rearrange("p (g j w) -> p g j w", g=G, j=2)
        vs3 = vsall_t[:].rearrange("p (b e) -> p b e", b=3)

        # MLP layer 1: h1 = relu(W0.T @ wT + b0) : [64, E]
        ps_h1 = ps_rot.tile([64, E], F32, tag="rot")
        nc.tensor.matmul(ps_h1[:], w0_t[:], wT_t[:], start=True, stop=True)
        h1_t = work.tile([64, E], BF16)
        nc.scalar.activation(
            h1_t[:], ps_h1[:], mybir.ActivationFunctionType.Relu,
            bias=b0c_t[:], scale=1.0,
        )

        # persistent PSUM accumulators
        psA = ps_acc.tile([128, E], F32, tag="A")
        psB = ps_acc.tile([64, E], F32, tag="B")
        psC = [ps_acc.tile([64, E], F32, tag=f"C{i}", name=f"psC{i}")
               for i in range(3)]
        psD = ps_acc.tile([128, E], F32, tag="D")

        # b2 contributions open each accumulation group
        nc.tensor.matmul(psA[:], ba_t[:], s1t_t[:], start=True, stop=False,
                         skip_group_check=True)
        nc.tensor.matmul(psB[:], bb_t[:], s1t_t[:], start=True, stop=False,
                         skip_group_check=True)
        for i in range(3):
            nc.tensor.matmul(psC[i][:], bc_t[:], vs3[:, i, :], start=True,
                             stop=False, skip_group_check=True)
        nc.tensor.matmul(psD[:], bd_t[:], d2t_t[:], start=True, stop=False,
                         skip_group_check=True)

        def bcast(g):
            ps_bc = ps_rot.tile([128, E], F32, tag="rot", name=f"bc{g}")
            nc.tensor.matmul(ps_bc[:], wg1_t[:, bass.ts(g, 128)], h1_t[:],
                             start=True, stop=True, skip_group_check=True)
            return ps_bc

        DR = mybir.MatmulPerfMode.DoubleRow
        pt8s = [None] * G  # retained fp8 product tiles for deferred residuals

        def emit_res(g, last):
            p2 = pt8s[g][:].rearrange("p (b e) -> p b e", b=4)
            for jp in range(2):
                nc.tensor.matmul(psA[:], wa8r4[:, g, jp], p2[:, 2 * jp:2 * jp + 2, :],
                                 start=False, stop=last and jp == 1,
                                 perf_mode=DR, skip_group_check=True)
            for jp in range(2):
                nc.tensor.matmul(psB[:], wb8r4[:, g, jp], p2[:, 2 * jp:2 * jp + 2, :],
                                 start=False, stop=last and jp == 1,
                                 perf_mode=DR, skip_group_check=True)

        ps_bc = bcast(0)
        for g in range(G):
            bct = bct_pool.tile([128, E], BF16, tag="bct")
            nc.scalar.activation(
                bct[:], ps_bc[:], mybir.ActivationFunctionType.Relu,
                bias=bg1_t[:, g: g + 1], scale=BETA,
            )
            # products: pt8 (A/B, fp8) and ptc (C/D, bf16)
            pt8 = pt8_pool.tile([128, 4 * E], FP8, tag="pt8")
            pt8s[g] = pt8
            bct_b4 = dataclasses.replace(
                bct[:], ap=[bct[:].ap[0], [0, 4], [1, E]]
            )
            nc.vector.scalar_tensor_tensor(
                pt8[:].rearrange("p (b e) -> p b e", b=4),
                f12[:, 0:4, :], 1.0, bct_b4,
                op0=mybir.AluOpType.mult, op1=mybir.AluOpType.mult,
            )
            ptc = ptc_pool.tile([128, 8 * E], BF16, tag="ptc")
            bct_b8 = dataclasses.replace(
                bct[:], ap=[bct[:].ap[0], [0, 8], [1, E]]
            )
            nc.vector.scalar_tensor_tensor(
                ptc[:].rearrange("p (b e) -> p b e", b=8),
                f12[:, 4:12, :], 1.0, bct_b8,
                op0=mybir.AluOpType.mult, op1=mybir.AluOpType.mult,
            )
            if g + 1 < G:
                ps_bc = bcast(g + 1)

            p2 = pt8[:].rearrange("p (b e) -> p b e", b=4)
            p8 = ptc[:].rearrange("p (b e) -> p b e", b=8)
            for jp in range(2):
                nc.tensor.matmul(psA[:], wa8m4[:, g, jp], p2[:, 2 * jp:2 * jp + 2, :],
                                 start=False, stop=False, perf_mode=DR,
                                 skip_group_check=True)
            for jp in range(2):
                nc.tensor.matmul(psB[:], wb8m4[:, g, jp], p2[:, 2 * jp:2 * jp + 2, :],
                                 start=False, stop=False, perf_mode=DR,
                                 skip_group_check=True)
            last = g == G - 1
            for i in range(3):
                for j in range(2):
                    nc.tensor.matmul(psC[i][:], wc3[:, g, j], p8[:, 2 * i + j, :],
                                     start=False, stop=last and j == 1,
                                     skip_group_check=True)
            for j in range(2):
                nc.tensor.matmul(psD[:], wd3[:, g, j], p8[:, 6 + j, :],
                                 start=False, stop=last and j == 1,
                                 skip_group_check=True)
            # deferred fp8 residual passes (weights arrive late; order-free)
            if g >= G // 2:
                emit_res(2 * (g - G // 2), last=False)
                emit_res(2 * (g - G // 2) + 1, last=last)

        # epilogue (still transposed):
        # out0T = psA*s2/(ab) + (I3/b)*psD + bias ; out1T_i = psB*v2_i/(ab) + psC_i/b
        tA = work.tile([128, E], F32)
        nc.vector.tensor_mul(tA[:], psA[:], s2b_t[:])
        tD = work.tile([128, E], F32)
        nc.scalar.mul(tD[:], psD[:], I3 / BETA)
        out0T = work.tile([128, E], F32)
        nc.vector.affine_then_add(out0T[:], tA[:], tD[:], scale=1.0,
                                  bias=bcol_t[:])
        out1T = []
        for i in range(3):
            tB = work.tile([64, E], F32, tag=f"tB{i}")
            nc.vector.tensor_mul(tB[:], psB[:], v2b_t[i][:])
            o1 = work.tile([64, E], F32, tag=f"o1{i}")
            nc.vector.affine_then_add(o1[:], psC[i][:], tB[:], scale=1.0 / BETA,
                                      bias=0.0)
            out1T.append(o1)

        # transpose back to [E, 320] and store
        for et in range(E // 128):
            sl = bass.ts(et, 128)
            outS = out_pool.tile([128, 320], F32, tag="outS")
            ps_t0 = ps_rot.tile([128, E], F32, tag="rot")
            nc.tensor.transpose(ps_t0[:, 0:128], out0T[:, sl], ident_t[:])
            nc.scalar.copy(outS[:, 0:128], ps_t0[:, 0:128])
            o1v = outS[:, 128:320].rearrange("p (w i) -> p i w", i=3)
            for i in range(3):
                ps_ti = ps_rot.tile([128, E], F32, tag="rot")
                nc.tensor.transpose(ps_ti[:, 0:64], out1T[i][:, sl],
                                    ident_t[0:64, 0:64])
                nc.scalar.copy(o1v[:, i, :], ps_ti[:, 0:64])
            nc.sync.dma_start(outp_d[sl, :], outS[:])

    nc.compile()
    return nc


_NC = None


def _get_nc():
    global _NC
    if _NC is None:
        _NC = _build_nc()
    return _NC


def _q8(x):
    return np.clip(x, -240.0, 240.0).astype(FP8_NP)


def _prep_inputs(data_in1, data_in2, weight, W0, b0, W1, b1, W2, b2, bias):
    f32 = np.float32
    data_in1 = np.ascontiguousarray(data_in1, dtype=f32)
    data_in2 = np.ascontiguousarray(data_in2, dtype=f32)
    weight = np.ascontiguousarray(weight, dtype=f32)
    W0 = np.asarray(W0, f32); b0 = np.asarray(b0, f32)
    W1 = np.asarray(W1, f32); b1 = np.asarray(b1, f32)
    W2 = np.asarray(W2, f32); b2 = np.asarray(b2, f32)
    bias = np.asarray(bias, f32)

    s1 = data_in1[:, :MUL0]                      # [N,128]
    v1 = data_in1[:, MUL0:].reshape(N, MUL1, 3)  # [N,64,3]
    s2 = data_in2[:, 0]                          # [N]
    v2 = data_in2[:, 1:4]                        # [N,3]

    def bf(x):
        return np.ascontiguousarray(x, dtype=f32).astype(BF16_NP)

    s1t = s1.T                                   # [128,N] f32
    dot12 = np.einsum("eui,ei->eu", v1, v2).T    # [64,N]
    vs = [(v1[:, :, i] * s2[:, None]).T for i in range(3)]  # [64,N] each

    # fused x-side blocks [128, 12, N]: 4 A/B + 6 C + 2 D
    blocks = []
    for j in range(4):
        blocks.append(np.tile(s1t[32 * j: 32 * j + 32], (4, 1)))
    for i in range(3):
        for j in range(2):
            blocks.append(np.tile(vs[i][32 * j: 32 * j + 32], (4, 1)))
    for j in range(2):
        blocks.append(np.tile(dot12[32 * j: 32 * j + 32], (4, 1)))
    fin0 = bf(np.stack(blocks, axis=1))          # [128, 12, N]

    wT = bf(weight.T)
    s2b = np.ascontiguousarray(
        np.broadcast_to(s2 * AB_INV, (128, N)), dtype=f32)
    v2b = [
        np.ascontiguousarray(np.broadcast_to(v2[:, i] * AB_INV, (64, N)),
                             dtype=f32)
        for i in range(3)
    ]

    # stationary chunk layouts: row r=(koff*32+uu) -> W[KPC*g+koff, 32*j+uu, :]
    def chunks(arr3):  # [64,U,W] -> [128(r), G, U//32(j), W]
        U, W = arr3.shape[1], arr3.shape[2]
        t = arr3.reshape(G, KPC, U // 32, 32, W)       # [g,koff,j,uu,w]
        return np.transpose(t, (1, 3, 0, 2, 4)).reshape(128, G, U // 32, W)

    Wa3 = W2[:, :N1].reshape(64, 128, 128)
    Wb3 = W2[:, N1:N1 + N2].reshape(64, 128, 64)
    Wc3 = W2[:, N1 + N2:N1 + N2 + N3].reshape(64, 64, 64)
    Wd3 = W2[:, N1 + N2 + N3:].reshape(64, 64, 128)

    # fp8 main + same-scale residual for A/B
    def fp8_pair(arr3):
        c = chunks(arr3) * ALPHA                       # [128, G, 4, W]
        m = _q8(c)
        r = _q8(c - m.astype(f32))
        return (np.ascontiguousarray(m).reshape(128, -1),
                np.ascontiguousarray(r).reshape(128, -1))

    wa8m, wa8r = fp8_pair(Wa3)
    wb8m, wb8r = fp8_pair(Wb3)

    shared = {
        "w0": bf(W0),
        "b0c": np.ascontiguousarray(b0.reshape(64, 1), f32),
        "wg1": bf(np.repeat(W1, 32, axis=1)),
        "bg1": np.ascontiguousarray(
            b1.reshape(G, KPC)[None, :, :].repeat(32, axis=0)
            .transpose(2, 0, 1).reshape(128, G) * BETA, f32),
        "wa8m": wa8m, "wa8r": wa8r, "wb8m": wb8m, "wb8r": wb8r,
        "wc": bf(chunks(Wc3).reshape(128, -1)),
        "wd": bf(chunks(Wd3).reshape(128, -1)),
        "ba": bf(b2[:N1].reshape(128, 128) * (ALPHA * BETA)),
        "bb": bf(b2[N1:N1 + N2].reshape(128, 64) * (ALPHA * BETA)),
        "bc": bf(b2[N1 + N2:N1 + N2 + N3].reshape(64, 64) * BETA),
        "bd": bf(b2[N1 + N2 + N3:].reshape(64, 128) * BETA),
        "bcol": np.ascontiguousarray(bias.reshape(128, 1), f32),
        "ident": np.eye(128, dtype=f32),
    }

    in_maps = []
    for c in range(N_CORES):
        e0 = c * E
        m = dict(shared)
        m["wT"] = np.ascontiguousarray(wT[:, e0:e0 + E])
        m["fin0"] = np.ascontiguousarray(
            fin0[:, :, e0:e0 + E]).reshape(128, 12 * E)
        m["s1t"] = bf(s1t[:, e0:e0 + E])
        m["vsall"] = bf(np.stack([v[:, e0:e0 + E] for v in vs], axis=1)
                        ).reshape(64, 3 * E)
        m["d2t"] = bf(dot12[:, e0:e0 + E])
        m["s2b"] = np.ascontiguousarray(s2b[:, e0:e0 + E])
        for i in range(3):
            m[f"v2b{i}"] = np.ascontiguousarray(v2b[i][:, e0:e0 + E])
        in_maps.append(m)
    return in_maps


def run(in_maps, **kwargs):
    nc = _get_nc()
    return run_bass_kernel_spmd(nc, in_maps, list(range(N_CORES)), **kwargs)


def kernel(data_in1, data_in2, weight, W0, b0, W1, b1, W2, b2, bias):
    in_maps = _prep_inputs(
        data_in1, data_in2, weight, W0, b0, W1, b1, W2, b2, bias
    )
    res = run(in_maps)
    out = np.concatenate(
        [np.asarray(res.results[c]["outp"]) for c in range(N_CORES)], axis=0
    )
    return out.astype(np.float32)


# revision 4
# speedup vs baseline: 1.7508x; 1.7508x over previous
"""Trainium2 Bass kernel for nn_O3TensorProductWeighted.

Computes, for each sample e:
    h  = relu(relu(weight @ W0 + b0) @ W1 + b1)           # [64]
    w  = h @ W2 + b2                                      # [36864] (never materialized)
    out0 = PW0*(einsum(Wa,s1)*s2 + I3*einsum(Wd,dot12))
    out1 = PW1*I3*(einsum(Wb,s1) x v2 + einsum(Wc,v1)*s2)
    out  = concat(out0, out1)/SQRT_K ; out[:128] += bias

Strategy: reassociate each einsum against the (k,u)-joint contraction of the
per-sample Khatri-Rao product h (x) x, so everything becomes dense matmuls
over shared W2 chunk weights, with the per-sample products built on-chip.
16 chunks of 4 k-values each; one PE broadcast matmul per chunk replicates
the 4 h2 rows 32x, ACT applies the relu (with a 2^2 scale folded in), and
tensor_mul ops (DVE 2x mode for bf16; Pool for most fp8 chunks) build the
per-sample products.

The A (0e x 0e) and B (0e x 1o) paths run as fp8-e4m3 DoubleRow matmuls
(two 128-row tiles summed per instruction at 0.5 cycles/row): weights are
split into an fp8 main + same-scale fp8 residual (two DR instructions) so
only the fp8 product rounding contributes error. C/D paths stay bf16.
Residual DR work is deferred to the second half of the chunk loop, and the
b2-bias matmuls run after the loop, so their DMAs never gate the PE; the
big weight streams are chunk-sliced so chunk 0 starts early. Scale
compensation (alpha*beta) is folded into host-prescaled epilogue operands.
Pure data parallel over 8 cores, transposed layout (features on
partitions, samples on the free dim).
"""

import dataclasses
import sys

sys.path.insert(0, "/opt/trn_rl_repo")

from contextlib import ExitStack

import ml_dtypes
import numpy as np

import concourse.bacc as bacc
import concourse.bass as bass
import concourse.tile as tile
from concourse import mybir
from concourse.bass_utils import run_bass_kernel_spmd

BF16 = mybir.dt.bfloat16
FP8 = mybir.dt.float8e4
F32 = mybir.dt.float32
BF16_NP = ml_dtypes.bfloat16
FP8_NP = ml_dtypes.float8_e4m3

N_CORES = 8
N = 4096
E = N // N_CORES  # 512 samples per core

MUL0, MUL1 = 128, 64
N1 = MUL0 * MUL0          # 16384
N2 = MUL0 * MUL1          # 8192
N3 = MUL1 * MUL1          # 4096
I3 = float(1.0 / np.sqrt(3.0))
# PW0/SQRT_K == 1.0 and PW1*I3/SQRT_K == 1.0 exactly; only I3 remains on D.

KPC = 4                   # k-values per chunk
G = 64 // KPC             # 16 chunks
NSL = 4                   # chunk-slices per big stationary stream
ALPHA = 1024.0            # fp8 scale on A/B stationary weights
BETA = 4.0                # scale folded into the relu -> rides on products
AB_INV = 1.0 / (ALPHA * BETA)
# chunks whose fp8 A/B products are built on Pool instead of DVE
POOL_AB = {0, 1, 3, 4, 6, 8, 9, 11, 12, 14}


def _build_nc():
    nc = bacc.Bacc(None)

    dp = nc.declare_dram_parameter
    wT_d = dp("wT", [16, E], BF16, isOutput=False)
    # x-side blocks: 4 A/B (s1 32-slices tiled 4x) ; 6 C (v1*s2) ; 2 D (dot12)
    finA_d = dp("finA", [128, 4 * E], BF16, isOutput=False)
    finCD_d = dp("finCD", [128, 8 * E], BF16, isOutput=False)
    s1t_d = dp("s1t", [128, E], BF16, isOutput=False)
    vsall_d = dp("vsall", [64, 3 * E], BF16, isOutput=False)
    d2t_d = dp("d2t", [64, E], BF16, isOutput=False)
    s2b_d = dp("s2b", [128, E], F32, isOutput=False)
    v2b_d = [dp(f"v2b{i}", [64, E], F32, isOutput=False) for i in range(3)]

    w0_d = dp("w0", [16, 64], BF16, isOutput=False)
    b0c_d = dp("b0c", [64, 1], F32, isOutput=False)
    wg1_d = dp("wg1", [64, G * 128], BF16, isOutput=False)
    bg1_d = dp("bg1", [128, G], F32, isOutput=False)
    # A/B stationaries: [128, G, 2(jp), 2(tile), W] fp8 main + residual
    wa8m_d = dp("wa8m", [128, G * 512], FP8, isOutput=False)
    wa8r_d = dp("wa8r", [128, G * 512], FP8, isOutput=False)
    wb8m_d = dp("wb8m", [128, G * 256], FP8, isOutput=False)
    wb8r_d = dp("wb8r", [128, G * 256], FP8, isOutput=False)
    # C/D stationaries: [128, G, 2(j), W] bf16
    wc_d = dp("wc", [128, G * 128], BF16, isOutput=False)
    wd_d = dp("wd", [128, G * 256], BF16, isOutput=False)
    ba_d = dp("ba", [128, 128], BF16, isOutput=False)
    bb_d = dp("bb", [128, 64], BF16, isOutput=False)
    bc_d = dp("bc", [64, 64], BF16, isOutput=False)
    bd_d = dp("bd", [64, 128], BF16, isOutput=False)
    bcol_d = dp("bcol", [128, 1], F32, isOutput=False)
    ident_d = dp("ident", [128, 128], F32, isOutput=False)

    outp_d = dp("outp", [E, 320], F32, isOutput=True)

    with tile.TileContext(nc) as tc, ExitStack() as ctx:
        const = ctx.enter_context(tc.tile_pool(name="const", bufs=1))
        work = ctx.enter_context(tc.tile_pool(name="work", bufs=1))
        bct_pool = ctx.enter_context(tc.tile_pool(name="bct", bufs=3))
        # pt8 tiles must survive ~8 chunks (residual DRs are deferred)
        pt8_pool = ctx.enter_context(tc.tile_pool(name="pt8", bufs=18))
        ptc_pool = ctx.enter_context(tc.tile_pool(name="ptc", bufs=3))
        out_pool = ctx.enter_context(tc.tile_pool(name="outs", bufs=2))
        ps_acc = ctx.enter_context(tc.tile_pool(name="ps_acc", bufs=1, space="PSUM"))
        ps_rot = ctx.enter_context(tc.tile_pool(name="ps_rot", bufs=2, space="PSUM"))

        def load(dparam, engine):
            t = const.tile(dparam.shape, dparam.dtype, name=f"t_{dparam.name}")
            engine.dma_start(t[:], dparam[:])
            return t

        def load_sliced(dparam, n):
            # n column-slices as separate tiles so early chunks start early;
            # DMAs are issued interleaved below
            w = dparam.shape[1] // n
            return [
                const.tile([dparam.shape[0], w], dparam.dtype,
                           name=f"t_{dparam.name}_{i}")
                for i in range(n)
            ], w

        # startup-critical small loads on Pool SWDGE
        wT_t = load(wT_d, nc.gpsimd)
        w0_t = load(w0_d, nc.gpsimd)
        b0c_t = load(b0c_d, nc.gpsimd)

        # big streams on SP HWDGE, chunk-sliced and interleaved by first use
        finA_t = load(finA_d, nc.sync)
        finCD_t = load(finCD_d, nc.sync)
        wg1_t = load(wg1_d, nc.sync)
        bg1_t = load(bg1_d, nc.sync)
        wa8m_ts, wa8m_w = load_sliced(wa8m_d, NSL)
        wb8m_ts, wb8m_w = load_sliced(wb8m_d, NSL)
        wc_ts, wc_w = load_sliced(wc_d, NSL)
        wd_ts, wd_w = load_sliced(wd_d, NSL)
        wa8r_ts, _ = load_sliced(wa8r_d, NSL)
        wb8r_ts, _ = load_sliced(wb8r_d, NSL)
        for i in range(NSL):
            for ts, d, w in ((wa8m_ts, wa8m_d, wa8m_w), (wb8m_ts, wb8m_d, wb8m_w),
                             (wc_ts, wc_d, wc_w), (wd_ts, wd_d, wd_w)):
                nc.sync.dma_start(ts[i][:], d[:, i * w: (i + 1) * w])
        for i in range(NSL):
            nc.sync.dma_start(wa8r_ts[i][:], wa8r_d[:, i * wa8m_w:(i + 1) * wa8m_w])
            nc.sync.dma_start(wb8r_ts[i][:], wb8r_d[:, i * wb8m_w:(i + 1) * wb8m_w])
        s2b_t = load(s2b_d, nc.sync)
        v2b_t = [load(d, nc.sync) for d in v2b_d]

        GS = G // NSL  # chunks per slice

        def wview(ts, g, jdim, w):  # [128, 2, jdim, w] views into slice tiles
            return ts[g // GS][:].rearrange(
                "p (g j t w) -> p g j t w", g=GS, j=2, t=jdim)[:, g % GS]

        fA = finA_t[:].rearrange("p (b e) -> p b e", b=4)
        fCD = finCD_t[:].rearrange("p (b e) -> p b e", b=8)

        # MLP layer 1: h1 = relu(W0.T @ wT + b0) : [64, E]
        ps_h1 = ps_rot.tile([64, E], F32, tag="rot")
        nc.tensor.matmul(ps_h1[:], w0_t[:], wT_t[:], start=True, stop=True)
        h1_t = work.tile([64, E], BF16)
        nc.scalar.activation(
            h1_t[:], ps_h1[:], mybir.ActivationFunctionType.Relu,
            bias=b0c_t[:], scale=1.0,
        )

        # persistent PSUM accumulators (groups opened by chunk-0 matmuls,
        # closed by the trailing b2-bias matmuls)
        psA = ps_acc.tile([128, E], F32, tag="A")
        psB = ps_acc.tile([64, E], F32, tag="B")
        psC = [ps_acc.tile([64, E], F32, tag=f"C{i}", name=f"psC{i}")
               for i in range(3)]
        psD = ps_acc.tile([128, E], F32, tag="D")

        def bcast(g):
            ps_bc = ps_rot.tile([128, E], F32, tag="rot", name=f"bc{g}")
            nc.tensor.matmul(ps_bc[:], wg1_t[:, bass.ts(g, 128)], h1_t[:],
                             start=True, stop=True, skip_group_check=True)
            return ps_bc

        DR = mybir.MatmulPerfMode.DoubleRow
        pt8s = [None] * G  # retained fp8 product tiles for deferred residuals

        def emit_res(g):
            p2 = pt8s[g][:].rearrange("p (b e) -> p b e", b=4)
            for jp in range(2):
                nc.tensor.matmul(psA[:], wview(wa8r_ts, g, 2, 128)[:, jp],
                                 p2[:, 2 * jp:2 * jp + 2, :],
                                 start=False, stop=False,
                                 perf_mode=DR, skip_group_check=True)
            for jp in range(2):
                nc.tensor.matmul(psB[:], wview(wb8r_ts, g, 2, 64)[:, jp],
                                 p2[:, 2 * jp:2 * jp + 2, :],
                                 start=False, stop=False,
                                 perf_mode=DR, skip_group_check=True)

        ps_bc = bcast(0)
        for g in range(G):
            bct = bct_pool.tile([128, E], BF16, tag="bct")
            nc.scalar.activation(
                bct[:], ps_bc[:], mybir.ActivationFunctionType.Relu,
                bias=bg1_t[:, g: g + 1], scale=BETA,
            )
            # products: pt8 (A/B, fp8) and ptc (C/D, bf16)
            pt8 = pt8_pool.tile([128, 4 * E], FP8, tag="pt8")
            pt8s[g] = pt8
            bct_b4 = dataclasses.replace(
                bct[:], ap=[bct[:].ap[0], [0, 4], [1, E]]
            )
            mul_eng = nc.gpsimd if g in POOL_AB else nc.vector
            mul_eng.tensor_mul(
                pt8[:].rearrange("p (b e) -> p b e", b=4), fA, bct_b4)
            ptc = ptc_pool.tile([128, 8 * E], BF16, tag="ptc")
            bct_b8 = dataclasses.replace(
                bct[:], ap=[bct[:].ap[0], [0, 8], [1, E]]
            )
            nc.vector.tensor_mul(
                ptc[:].rearrange("p (b e) -> p b e", b=8), fCD, bct_b8)
            if g + 1 < G:
                ps_bc = bcast(g + 1)

            p2 = pt8[:].rearrange("p (b e) -> p b e", b=4)
            p8 = ptc[:].rearrange("p (b e) -> p b e", b=8)
            for jp in range(2):
                nc.tensor.matmul(psA[:], wview(wa8m_ts, g, 2, 128)[:, jp],
                                 p2[:, 2 * jp:2 * jp + 2, :],
                                 start=(g == 0 and jp == 0), stop=False,
                                 perf_mode=DR, skip_group_check=True)
            for jp in range(2):
                nc.tensor.matmul(psB[:], wview(wb8m_ts, g, 2, 64)[:, jp],
                                 p2[:, 2 * jp:2 * jp + 2, :],
                                 start=(g == 0 and jp == 0), stop=False,
                                 perf_mode=DR, skip_group_check=True)
            for i in range(3):
                for j in range(2):
                    nc.tensor.matmul(psC[i][:], wview(wc_ts, g, 1, 64)[:, j],
                                     p8[:, 2 * i + j, :],
                                     start=(g == 0 and j == 0), stop=False,
                                     skip_group_check=True)
            for j in range(2):
                nc.tensor.matmul(psD[:], wview(wd_ts, g, 1, 128)[:, j],
                                 p8[:, 6 + j, :],
                                 start=(g == 0 and j == 0), stop=False,
                                 skip_group_check=True)
            # deferred fp8 residual passes (weights arrive late; order-free)
            if g >= G // 2:
                emit_res(2 * (g - G // 2))
                emit_res(2 * (g - G // 2) + 1)

        # late small loads on ACT HWDGE (queued behind the loop's relus)
        s1t_t = load(s1t_d, nc.scalar)
        vsall_t2 = load(vsall_d, nc.scalar)
        d2t_t = load(d2t_d, nc.scalar)
        ba_t = load(ba_d, nc.scalar)
        bb_t = load(bb_d, nc.scalar)
        bc_t = load(bc_d, nc.scalar)
        bd_t = load(bd_d, nc.scalar)
        bcol_t = load(bcol_d, nc.scalar)
        ident_t = load(ident_d, nc.scalar)
        vs3 = vsall_t2[:].rearrange("p (b e) -> p b e", b=3)

        # b2 contributions close each accumulation group
        nc.tensor.matmul(psA[:], ba_t[:], s1t_t[:], start=False, stop=True,
                         skip_group_check=True)
        nc.tensor.matmul(psB[:], bb_t[:], s1t_t[:], start=False, stop=True,
                         skip_group_check=True)
        for i in range(3):
            nc.tensor.matmul(psC[i][:], bc_t[:], vs3[:, i, :], start=False,
                             stop=True, skip_group_check=True)
        nc.tensor.matmul(psD[:], bd_t[:], d2t_t[:], start=False, stop=True,
                         skip_group_check=True)

        # epilogue (still transposed):
        # out0T = psA*s2/(ab) + (I3/b)*psD + bias ; out1T_i = psB*v2_i/(ab) + psC_i/b
        tA = work.tile([128, E], F32)
        nc.vector.tensor_mul(tA[:], psA[:], s2b_t[:])
        tD = work.tile([128, E], F32)
        nc.scalar.mul(tD[:], psD[:], I3 / BETA)
        out0T = work.tile([128, E], F32)
        nc.vector.affine_then_add(out0T[:], tA[:], tD[:], scale=1.0,
                                  bias=bcol_t[:])
        out1T = []
        for i in range(3):
            tB = work.tile([64, E], F32, tag=f"tB{i}")
            nc.vector.tensor_mul(tB[:], psB[:], v2b_t[i][:])
            o1 = work.tile([64, E], F32, tag=f"o1{i}")
            nc.vector.affine_then_add(o1[:], psC[i][:], tB[:], scale=1.0 / BETA,
                                      bias=0.0)
            out1T.append(o1)

        # transpose back to [E, 320] and store
        for et in range(E // 128):
            sl = bass.ts(et, 128)
            outS = out_pool.tile([128, 320], F32, tag="outS")
            ps_t0 = ps_rot.tile([128, E], F32, tag="rot")
            nc.tensor.transpose(ps_t0[:, 0:128], out0T[:, sl], ident_t[:])
            nc.scalar.copy(outS[:, 0:128], ps_t0[:, 0:128])
            o1v = outS[:, 128:320].rearrange("p (w i) -> p i w", i=3)
            for i in range(3):
                ps_ti = ps_rot.tile([128, E], F32, tag="rot")
                nc.tensor.transpose(ps_ti[:, 0:64], out1T[i][:, sl],
                                    ident_t[0:64, 0:64])
                nc.scalar.copy(o1v[:, i, :], ps_ti[:, 0:64])
            nc.sync.dma_start(outp_d[sl, :], outS[:])

    nc.compile()
    return nc


_NC = None


def _get_nc():
    global _NC
    if _NC is None:
        _NC = _build_nc()
    return _NC


def _q8(x):
    return np.clip(x, -240.0, 240.0).astype(FP8_NP)


def _prep_inputs(data_in1, data_in2, weight, W0, b0, W1, b1, W2, b2, bias):
    f32 = np.float32
    data_in1 = np.ascontiguousarray(data_in1, dtype=f32)
    data_in2 = np.ascontiguousarray(data_in2, dtype=f32)
    weight = np.ascontiguousarray(weight, dtype=f32)
    W0 = np.asarray(W0, f32); b0 = np.asarray(b0, f32)
    W1 = np.asarray(W1, f32); b1 = np.asarray(b1, f32)
    W2 = np.asarray(W2, f32); b2 = np.asarray(b2, f32)
    bias = np.asarray(bias, f32)

    s1 = data_in1[:, :MUL0]                      # [N,128]
    v1 = data_in1[:, MUL0:].reshape(N, MUL1, 3)  # [N,64,3]
    s2 = data_in2[:, 0]                          # [N]
    v2 = data_in2[:, 1:4]                        # [N,3]

    def bf(x):
        return np.ascontiguousarray(x, dtype=f32).astype(BF16_NP)

    s1t = s1.T                                   # [128,N] f32
    dot12 = np.einsum("eui,ei->eu", v1, v2).T    # [64,N]
    vs = [(v1[:, :, i] * s2[:, None]).T for i in range(3)]  # [64,N] each

    # x-side blocks: A/B [128, 4, N], C/D [128, 8, N]
    blkA = [np.tile(s1t[32 * j: 32 * j + 32], (4, 1)) for j in range(4)]
    blkCD = []
    for i in range(3):
        for j in range(2):
            blkCD.append(np.tile(vs[i][32 * j: 32 * j + 32], (4, 1)))
    for j in range(2):
        blkCD.append(np.tile(dot12[32 * j: 32 * j + 32], (4, 1)))
    finA = bf(np.stack(blkA, axis=1))            # [128, 4, N]
    finCD = bf(np.stack(blkCD, axis=1))          # [128, 8, N]

    wT = bf(weight.T)
    s2b = np.ascontiguousarray(
        np.broadcast_to(s2 * AB_INV, (128, N)), dtype=f32)
    v2b = [
        np.ascontiguousarray(np.broadcast_to(v2[:, i] * AB_INV, (64, N)),
                             dtype=f32)
        for i in range(3)
    ]

    # stationary chunk layouts: row r=(koff*32+uu) -> W[KPC*g+koff, 32*j+uu, :]
    def chunks(arr3):  # [64,U,W] -> [128(r), G, U//32(j), W]
        U, W = arr3.shape[1], arr3.shape[2]
        t = arr3.reshape(G, KPC, U // 32, 32, W)       # [g,koff,j,uu,w]
        return np.transpose(t, (1, 3, 0, 2, 4)).reshape(128, G, U // 32, W)

    Wa3 = W2[:, :N1].reshape(64, 128, 128)
    Wb3 = W2[:, N1:N1 + N2].reshape(64, 128, 64)
    Wc3 = W2[:, N1 + N2:N1 + N2 + N3].reshape(64, 64, 64)
    Wd3 = W2[:, N1 + N2 + N3:].reshape(64, 64, 128)

    # fp8 main + same-scale residual for A/B
    def fp8_pair(arr3):
        c = chunks(arr3) * ALPHA                       # [128, G, 4, W]
        m = _q8(c)
        r = _q8(c - m.astype(f32))
        return (np.ascontiguousarray(m).reshape(128, -1),
                np.ascontiguousarray(r).reshape(128, -1))

    wa8m, wa8r = fp8_pair(Wa3)
    wb8m, wb8r = fp8_pair(Wb3)

    shared = {
        "w0": bf(W0),
        "b0c": np.ascontiguousarray(b0.reshape(64, 1), f32),
        "wg1": bf(np.repeat(W1, 32, axis=1)),
        "bg1": np.ascontiguousarray(
            b1.reshape(G, KPC)[None, :, :].repeat(32, axis=0)
            .transpose(2, 0, 1).reshape(128, G) * BETA, f32),
        "wa8m": wa8m, "wa8r": wa8r, "wb8m": wb8m, "wb8r": wb8r,
        "wc": bf(chunks(Wc3).reshape(128, -1)),
        "wd": bf(chunks(Wd3).reshape(128, -1)),
        "ba": bf(b2[:N1].reshape(128, 128) * (ALPHA * BETA)),
        "bb": bf(b2[N1:N1 + N2].reshape(128, 64) * (ALPHA * BETA)),
        "bc": bf(b2[N1 + N2:N1 + N2 + N3].reshape(64, 64) * BETA),
        "bd": bf(b2[N1 + N2 + N3:].reshape(64, 128) * BETA),
        "bcol": np.ascontiguousarray(bias.reshape(128, 1), f32),
        "ident": np.eye(128, dtype=f32),
    }

    in_maps = []
    for c in range(N_CORES):
        e0 = c * E
        m = dict(shared)
        m["wT"] = np.ascontiguousarray(wT[:, e0:e0 + E])
        m["finA"] = np.ascontiguousarray(
            finA[:, :, e0:e0 + E]).reshape(128, 4 * E)
        m["finCD"] = np.ascontiguousarray(
            finCD[:, :, e0:e0 + E]).reshape(128, 8 * E)
        m["s1t"] = bf(s1t[:, e0:e0 + E])
        m["vsall"] = bf(np.stack([v[:, e0:e0 + E] for v in vs], axis=1)
                        ).reshape(64, 3 * E)
        m["d2t"] = bf(dot12[:, e0:e0 + E])
        m["s2b"] = np.ascontiguousarray(s2b[:, e0:e0 + E])
        for i in range(3):
            m[f"v2b{i}"] = np.ascontiguousarray(v2b[i][:, e0:e0 + E])
        in_maps.append(m)
    return in_maps


def run(in_maps, **kwargs):
    nc = _get_nc()
    return run_bass_kernel_spmd(nc, in_maps, list(range(N_CORES)), **kwargs)


def kernel(data_in1, data_in2, weight, W0, b0, W1, b1, W2, b2, bias):
    in_maps = _prep_inputs(
        data_in1, data_in2, weight, W0, b0, W1, b1, W2, b2, bias
    )
    res = run(in_maps)
    out = np.concatenate(
        [np.asarray(res.results[c]["outp"]) for c in range(N_CORES)], axis=0
    )
    return out.astype(np.float32)


# revision 5
# speedup vs baseline: 1.7516x; 1.0005x over previous
"""Trainium2 Bass kernel for nn_O3TensorProductWeighted.

Computes, for each sample e:
    h  = relu(relu(weight @ W0 + b0) @ W1 + b1)           # [64]
    w  = h @ W2 + b2                                      # [36864] (never materialized)
    out0 = PW0*(einsum(Wa,s1)*s2 + I3*einsum(Wd,dot12))
    out1 = PW1*I3*(einsum(Wb,s1) x v2 + einsum(Wc,v1)*s2)
    out  = concat(out0, out1)/SQRT_K ; out[:128] += bias

Strategy: reassociate each einsum against the (k,u)-joint contraction of the
per-sample Khatri-Rao product h (x) x, so everything becomes dense matmuls
over shared W2 chunk weights, with the per-sample products built on-chip.
8 chunks of 8 k-values each; one PE broadcast matmul per chunk replicates
the 8 h2 rows 16x, ACT applies the relu (with a 2^2 scale folded in), and
tensor_mul ops (Pool for fp8 A/B products, DVE 2x mode for bf16 C/D) build
the per-sample Khatri-Rao products.

The A (0e x 0e) and B (0e x 1o) paths run as fp8-e4m3 DoubleRow matmuls
(two 128-row tiles summed per instruction at 0.5 cycles/row): weights are
split into an fp8 main + same-scale fp8 residual (two DR instructions) so
only the fp8 product rounding contributes error. C/D paths stay bf16.
Residual DR work is deferred to the second half of the chunk loop and the
b2-bias matmuls run mid-loop, so no DMA ever gates the PE; the big weight
streams are chunk-sliced and interleaved so chunk 0 starts early. Scale
compensation (alpha*beta) is folded into host-prescaled epilogue operands.
The epilogue runs in two sample-halves so DVE work overlaps the PE
transposes and output DMA. Pure data parallel over 8 cores, transposed
layout (features on partitions, samples on the free dim).
"""

import dataclasses
import sys

sys.path.insert(0, "/opt/trn_rl_repo")

from contextlib import ExitStack

import ml_dtypes
import numpy as np

import concourse.bacc as bacc
import concourse.bass as bass
import concourse.tile as tile
from concourse import mybir
from concourse.bass_utils import run_bass_kernel_spmd

BF16 = mybir.dt.bfloat16
FP8 = mybir.dt.float8e4
F32 = mybir.dt.float32
BF16_NP = ml_dtypes.bfloat16
FP8_NP = ml_dtypes.float8_e4m3

N_CORES = 8
N = 4096
E = N // N_CORES  # 512 samples per core

MUL0, MUL1 = 128, 64
N1 = MUL0 * MUL0          # 16384
N2 = MUL0 * MUL1          # 8192
N3 = MUL1 * MUL1          # 4096
I3 = float(1.0 / np.sqrt(3.0))
# PW0/SQRT_K == 1.0 and PW1*I3/SQRT_K == 1.0 exactly; only I3 remains on D.

KPC = 8                   # k-values per chunk
G = 64 // KPC             # 8 chunks
TPB = 128 // KPC          # 16 u-values per row-block
NSL = 4                   # chunk-slices per big stationary stream
NJA = 128 // TPB // 2     # 4 DoubleRow pairs per chunk on A/B
NJC = 64 // TPB           # 4 row-blocks per chunk on C/D
ALPHA = 1024.0            # fp8 scale on A/B stationary weights
BETA = 4.0                # scale folded into the relu -> rides on products
AB_INV = 1.0 / (ALPHA * BETA)
POOL_CD = {3}             # chunks whose C/D products build on Pool (A/B: all)


def _build_nc():
    nc = bacc.Bacc(None)

    dp = nc.declare_dram_parameter
    wT_d = dp("wT", [16, E], BF16, isOutput=False)
    # x-side blocks: 8 A/B (s1 16-slices tiled 8x) ; 12 C (v1*s2) ; 4 D (dot12)
    finA_d = dp("finA", [128, 8 * E], BF16, isOutput=False)
    finCD_d = dp("finCD", [128, 16 * E], BF16, isOutput=False)
    s1t_d = dp("s1t", [128, E], BF16, isOutput=False)
    vsall_d = dp("vsall", [64, 3 * E], BF16, isOutput=False)
    d2t_d = dp("d2t", [64, E], BF16, isOutput=False)
    s2b_d = dp("s2b", [128, E], F32, isOutput=False)
    v2b_d = [dp(f"v2b{i}", [64, E], F32, isOutput=False) for i in range(3)]

    w0_d = dp("w0", [16, 64], BF16, isOutput=False)
    b0c_d = dp("b0c", [64, 1], F32, isOutput=False)
    wg1_d = dp("wg1", [64, G * 128], BF16, isOutput=False)
    bg1_d = dp("bg1", [128, G], F32, isOutput=False)
    # A/B stationaries: [128, G, NJA(jp), 2(tile), W] fp8 main + residual
    wa8m_d = dp("wa8m", [128, G * NJA * 2 * 128], FP8, isOutput=False)
    wa8r_d = dp("wa8r", [128, G * NJA * 2 * 128], FP8, isOutput=False)
    wb8m_d = dp("wb8m", [128, G * NJA * 2 * 64], FP8, isOutput=False)
    wb8r_d = dp("wb8r", [128, G * NJA * 2 * 64], FP8, isOutput=False)
    # C/D stationaries: [128, G, NJC(j), W] bf16
    wc_d = dp("wc", [128, G * NJC * 64], BF16, isOutput=False)
    wd_d = dp("wd", [128, G * NJC * 128], BF16, isOutput=False)
    ba_d = dp("ba", [128, 128], BF16, isOutput=False)
    bb_d = dp("bb", [128, 64], BF16, isOutput=False)
    bc_d = dp("bc", [64, 64], BF16, isOutput=False)
    bd_d = dp("bd", [64, 128], BF16, isOutput=False)
    bcol_d = dp("bcol", [128, 1], F32, isOutput=False)
    ident_d = dp("ident", [128, 128], F32, isOutput=False)

    outp_d = dp("outp", [E, 320], F32, isOutput=True)

    with tile.TileContext(nc) as tc, ExitStack() as ctx:
        const = ctx.enter_context(tc.tile_pool(name="const", bufs=1))
        work = ctx.enter_context(tc.tile_pool(name="work", bufs=1))
        bct_pool = ctx.enter_context(tc.tile_pool(name="bct", bufs=3))
        # pt8 tiles must survive ~4 chunks (residual DRs are deferred)
        pt8_pool = ctx.enter_context(tc.tile_pool(name="pt8", bufs=10))
        ptc_pool = ctx.enter_context(tc.tile_pool(name="ptc", bufs=3))
        out_pool = ctx.enter_context(tc.tile_pool(name="outs", bufs=2))
        ps_acc = ctx.enter_context(tc.tile_pool(name="ps_acc", bufs=1, space="PSUM"))
        ps_rot = ctx.enter_context(tc.tile_pool(name="ps_rot", bufs=2, space="PSUM"))

        def load(dparam, engine=None):
            t = const.tile(dparam.shape, dparam.dtype, name=f"t_{dparam.name}")
            (engine or nc.sync).dma_start(t[:], dparam[:])
            return t

        def sliced_tiles(dparam, n):
            w = dparam.shape[1] // n
            return [
                const.tile([dparam.shape[0], w], dparam.dtype,
                           name=f"t_{dparam.name}_{i}")
                for i in range(n)
            ], w

        # everything on SP HWDGE, ordered by first use; big stationary
        # streams are chunk-sliced with residuals interleaved per round
        wT_t = load(wT_d)
        w0_t = load(w0_d)
        b0c_t = load(b0c_d)
        wg1_t = load(wg1_d)
        bg1_t = load(bg1_d)
        finA_t = load(finA_d)
        finCD_t = load(finCD_d)
        wa8m_ts, wa8m_w = sliced_tiles(wa8m_d, NSL)
        wb8m_ts, wb8m_w = sliced_tiles(wb8m_d, NSL)
        wc_ts, wc_w = sliced_tiles(wc_d, NSL)
        wd_ts, wd_w = sliced_tiles(wd_d, NSL)
        wa8r_ts, _ = sliced_tiles(wa8r_d, NSL)
        wb8r_ts, _ = sliced_tiles(wb8r_d, NSL)
        for i in range(NSL):
            for ts, d, w in ((wa8m_ts, wa8m_d, wa8m_w), (wb8m_ts, wb8m_d, wb8m_w),
                             (wc_ts, wc_d, wc_w), (wd_ts, wd_d, wd_w),
                             (wa8r_ts, wa8r_d, wa8m_w), (wb8r_ts, wb8r_d, wb8m_w)):
                nc.sync.dma_start(ts[i][:], d[:, i * w: (i + 1) * w])
        s1t_t = load(s1t_d)
        vsall_t = load(vsall_d)
        d2t_t = load(d2t_d)
        ba_t = load(ba_d)
        bb_t = load(bb_d)
        bc_t = load(bc_d)
        bd_t = load(bd_d)
        s2b_t = load(s2b_d)
        v2b_t = [load(d) for d in v2b_d]
        bcol_t = load(bcol_d)
        ident_t = load(ident_d)

        GS = G // NSL  # chunks per slice

        def wview(ts, g, jdim, tdim, w):
            return ts[g // GS][:].rearrange(
                "p (g j t w) -> p g j t w", g=GS, j=jdim, t=tdim)[:, g % GS]

        fA = finA_t[:].rearrange("p (b e) -> p b e", b=8)
        fCD = finCD_t[:].rearrange("p (b e) -> p b e", b=16)
        vs3 = vsall_t[:].rearrange("p (b e) -> p b e", b=3)

        # MLP layer 1: h1 = relu(W0.T @ wT + b0) : [64, E]
        ps_h1 = ps_rot.tile([64, E], F32, tag="rot")
        nc.tensor.matmul(ps_h1[:], w0_t[:], wT_t[:], start=True, stop=True)
        h1_t = work.tile([64, E], BF16)
        nc.scalar.activation(
            h1_t[:], ps_h1[:], mybir.ActivationFunctionType.Relu,
            bias=b0c_t[:], scale=1.0,
        )

        # persistent PSUM accumulators (groups opened by chunk-0 matmuls,
        # closed by the final deferred residual / last-chunk C,D matmuls)
        psA = ps_acc.tile([128, E], F32, tag="A")
        psB = ps_acc.tile([64, E], F32, tag="B")
        psC = [ps_acc.tile([64, E], F32, tag=f"C{i}", name=f"psC{i}")
               for i in range(3)]
        psD = ps_acc.tile([128, E], F32, tag="D")

        def bcast(g):
            ps_bc = ps_rot.tile([128, E], F32, tag="rot", name=f"bc{g}")
            nc.tensor.matmul(ps_bc[:], wg1_t[:, bass.ts(g, 128)], h1_t[:],
                             start=True, stop=True, skip_group_check=True)
            return ps_bc

        DR = mybir.MatmulPerfMode.DoubleRow
        pt8s = [None] * G  # retained fp8 product tiles for deferred residuals

        def emit_res(g, stop):
            p2 = pt8s[g][:].rearrange("p (b e) -> p b e", b=2 * NJA)
            wa = wview(wa8r_ts, g, NJA, 2, 128)
            wb = wview(wb8r_ts, g, NJA, 2, 64)
            for jp in range(NJA):
                nc.tensor.matmul(psA[:], wa[:, jp], p2[:, 2 * jp:2 * jp + 2, :],
                                 start=False, stop=stop and jp == NJA - 1,
                                 perf_mode=DR, skip_group_check=True)
            for jp in range(NJA):
                nc.tensor.matmul(psB[:], wb[:, jp], p2[:, 2 * jp:2 * jp + 2, :],
                                 start=False, stop=stop and jp == NJA - 1,
                                 perf_mode=DR, skip_group_check=True)

        ps_bc = bcast(0)
        for g in range(G):
            bct = bct_pool.tile([128, E], BF16, tag="bct")
            nc.scalar.activation(
                bct[:], ps_bc[:], mybir.ActivationFunctionType.Relu,
                bias=bg1_t[:, g: g + 1], scale=BETA,
            )
            # products: pt8 (A/B on Pool, fp8) and ptc (C/D, bf16)
            pt8 = pt8_pool.tile([128, 2 * NJA * E], FP8, tag="pt8")
            pt8s[g] = pt8
            bct_b = dataclasses.replace(
                bct[:], ap=[bct[:].ap[0], [0, 2 * NJA], [1, E]]
            )
            nc.gpsimd.tensor_mul(
                pt8[:].rearrange("p (b e) -> p b e", b=2 * NJA), fA, bct_b)
            ptc = ptc_pool.tile([128, 4 * NJC * E], BF16, tag="ptc")
            bct_c = dataclasses.replace(
                bct[:], ap=[bct[:].ap[0], [0, 4 * NJC], [1, E]]
            )
            cd_eng = nc.gpsimd if g in POOL_CD else nc.vector
            cd_eng.tensor_mul(
                ptc[:].rearrange("p (b e) -> p b e", b=4 * NJC), fCD, bct_c)
            if g + 1 < G:
                ps_bc = bcast(g + 1)

            p2 = pt8[:].rearrange("p (b e) -> p b e", b=2 * NJA)
            p8 = ptc[:].rearrange("p (b e) -> p b e", b=4 * NJC)
            wa = wview(wa8m_ts, g, NJA, 2, 128)
            wb = wview(wb8m_ts, g, NJA, 2, 64)
            wcv = wview(wc_ts, g, NJC, 1, 64)
            wdv = wview(wd_ts, g, NJC, 1, 128)
            for jp in range(NJA):
                nc.tensor.matmul(psA[:], wa[:, jp], p2[:, 2 * jp:2 * jp + 2, :],
                                 start=(g == 0 and jp == 0), stop=False,
                                 perf_mode=DR, skip_group_check=True)
            for jp in range(NJA):
                nc.tensor.matmul(psB[:], wb[:, jp], p2[:, 2 * jp:2 * jp + 2, :],
                                 start=(g == 0 and jp == 0), stop=False,
                                 perf_mode=DR, skip_group_check=True)
            last = g == G - 1
            for i in range(3):
                for j in range(NJC):
                    nc.tensor.matmul(psC[i][:], wcv[:, j], p8[:, NJC * i + j, :],
                                     start=(g == 0 and j == 0),
                                     stop=last and j == NJC - 1,
                                     skip_group_check=True)
            for j in range(NJC):
                nc.tensor.matmul(psD[:], wdv[:, j], p8[:, 3 * NJC + j, :],
                                 start=(g == 0 and j == 0),
                                 stop=last and j == NJC - 1,
                                 skip_group_check=True)
            if g == G // 2:
                # b2 contributions ride mid-loop (their DMAs arrived by now)
                nc.tensor.matmul(psA[:], ba_t[:], s1t_t[:], start=False,
                                 stop=False, skip_group_check=True)
                nc.tensor.matmul(psB[:], bb_t[:], s1t_t[:], start=False,
                                 stop=False, skip_group_check=True)
                for i in range(3):
                    nc.tensor.matmul(psC[i][:], bc_t[:], vs3[:, i, :],
                                     start=False, stop=False,
                                     skip_group_check=True)
                nc.tensor.matmul(psD[:], bd_t[:], d2t_t[:], start=False,
                                 stop=False, skip_group_check=True)
            # deferred fp8 residual passes (weights arrive late; order-free)
            if g >= G // 2:
                emit_res(2 * (g - G // 2), stop=False)
                emit_res(2 * (g - G // 2) + 1, stop=last)

        # epilogue in two sample-halves so DVE overlaps PE transposes + DMA:
        # out0T = psA*s2/(ab) + (I3/b)*psD + bias ; out1T_i = psB*v2_i/(ab) + psC_i/b
        out0T = work.tile([128, E], F32)
        out1T = [work.tile([64, E], F32, tag=f"o1{i}", name=f"o1{i}")
                 for i in range(3)]
        for h in range(2):
            sl = bass.ts(h, E // 2)
            tA = work.tile([128, E // 2], F32, tag=f"tA{h}", name=f"tA{h}")
            nc.vector.tensor_mul(tA[:], psA[:, sl], s2b_t[:, sl])
            tD = work.tile([128, E // 2], F32, tag=f"tD{h}", name=f"tD{h}")
            nc.scalar.mul(tD[:], psD[:, sl], I3 / BETA)
            nc.vector.affine_then_add(out0T[:, sl], tA[:], tD[:], scale=1.0,
                                      bias=bcol_t[:])
            for i in range(3):
                tB = work.tile([64, E // 2], F32, tag=f"tB{i}{h}",
                               name=f"tB{i}{h}")
                nc.vector.tensor_mul(tB[:], psB[:, sl], v2b_t[i][:, sl])
                nc.vector.affine_then_add(out1T[i][:, sl], psC[i][:, sl],
                                          tB[:], scale=1.0 / BETA, bias=0.0)
            # transpose this half back to [E, 320] and store
            for eh in range(2):
                et = 2 * h + eh
                slt = bass.ts(et, 128)
                outS = out_pool.tile([128, 320], F32, tag="outS")
                ps_t0 = ps_rot.tile([128, E], F32, tag="rot")
                nc.tensor.transpose(ps_t0[:, 0:128], out0T[:, slt], ident_t[:])
                nc.scalar.copy(outS[:, 0:128], ps_t0[:, 0:128])
                o1v = outS[:, 128:320].rearrange("p (w i) -> p i w", i=3)
                for i in range(3):
                    ps_ti = ps_rot.tile([128, E], F32, tag="rot")
                    nc.tensor.transpose(ps_ti[:, 0:64], out1T[i][:, slt],
                                        ident_t[0:64, 0:64])
                    nc.scalar.copy(o1v[:, i, :], ps_ti[:, 0:64])
                nc.sync.dma_start(outp_d[slt, :], outS[:])

    nc.compile()
    return nc


_NC = None


def _get_nc():
    global _NC
    if _NC is None:
        _NC = _build_nc()
    return _NC


def _q8(x):
    return np.clip(x, -240.0, 240.0).astype(FP8_NP)


def _prep_inputs(data_in1, data_in2, weight, W0, b0, W1, b1, W2, b2, bias):
    f32 = np.float32
    data_in1 = np.ascontiguousarray(data_in1, dtype=f32)
    data_in2 = np.ascontiguousarray(data_in2, dtype=f32)
    weight = np.ascontiguousarray(weight, dtype=f32)
    W0 = np.asarray(W0, f32); b0 = np.asarray(b0, f32)
    W1 = np.asarray(W1, f32); b1 = np.asarray(b1, f32)
    W2 = np.asarray(W2, f32); b2 = np.asarray(b2, f32)
    bias = np.asarray(bias, f32)

    s1 = data_in1[:, :MUL0]                      # [N,128]
    v1 = data_in1[:, MUL0:].reshape(N, MUL1, 3)  # [N,64,3]
    s2 = data_in2[:, 0]                          # [N]
    v2 = data_in2[:, 1:4]                        # [N,3]

    def bf(x):
        return np.ascontiguousarray(x, dtype=f32).astype(BF16_NP)

    s1t = s1.T                                   # [128,N] f32
    dot12 = np.einsum("eui,ei->eu", v1, v2).T    # [64,N]
    vs = [(v1[:, :, i] * s2[:, None]).T for i in range(3)]  # [64,N] each

    # x-side blocks: A/B [128, 8, N], C/D [128, 16, N]
    blkA = [np.tile(s1t[TPB * j: TPB * (j + 1)], (KPC, 1)) for j in range(8)]
    blkCD = []
    for i in range(3):
        for j in range(NJC):
            blkCD.append(np.tile(vs[i][TPB * j: TPB * (j + 1)], (KPC, 1)))
    for j in range(NJC):
        blkCD.append(np.tile(dot12[TPB * j: TPB * (j + 1)], (KPC, 1)))
    finA = bf(np.stack(blkA, axis=1))            # [128, 8, N]
    finCD = bf(np.stack(blkCD, axis=1))          # [128, 16, N]

    wT = bf(weight.T)
    s2b = np.ascontiguousarray(
        np.broadcast_to(s2 * AB_INV, (128, N)), dtype=f32)
    v2b = [
        np.ascontiguousarray(np.broadcast_to(v2[:, i] * AB_INV, (64, N)),
                             dtype=f32)
        for i in range(3)
    ]

    # stationary chunk layouts: row r=(koff*TPB+uu) -> W[KPC*g+koff, TPB*j+uu, :]
    def chunks(arr3):  # [64,U,W] -> [128(r), G, U//TPB(j), W]
        U, W = arr3.shape[1], arr3.shape[2]
        t = arr3.reshape(G, KPC, U // TPB, TPB, W)     # [g,koff,j,uu,w]
        return np.transpose(t, (1, 3, 0, 2, 4)).reshape(128, G, U // TPB, W)

    Wa3 = W2[:, :N1].reshape(64, 128, 128)
    Wb3 = W2[:, N1:N1 + N2].reshape(64, 128, 64)
    Wc3 = W2[:, N1 + N2:N1 + N2 + N3].reshape(64, 64, 64)
    Wd3 = W2[:, N1 + N2 + N3:].reshape(64, 64, 128)

    # fp8 main + same-scale residual for A/B
    def fp8_pair(arr3):
        c = chunks(arr3) * ALPHA                       # [128, G, U//TPB, W]
        m = _q8(c)
        r = _q8(c - m.astype(f32))
        return (np.ascontiguousarray(m).reshape(128, -1),
                np.ascontiguousarray(r).reshape(128, -1))

    wa8m, wa8r = fp8_pair(Wa3)
    wb8m, wb8r = fp8_pair(Wb3)

    shared = {
        "w0": bf(W0),
        "b0c": np.ascontiguousarray(b0.reshape(64, 1), f32),
        "wg1": bf(np.repeat(W1, TPB, axis=1)),
        "bg1": np.ascontiguousarray(
            b1.reshape(G, KPC)[None, :, :].repeat(TPB, axis=0)
            .transpose(2, 0, 1).reshape(128, G) * BETA, f32),
        "wa8m": wa8m, "wa8r": wa8r, "wb8m": wb8m, "wb8r": wb8r,
        "wc": bf(chunks(Wc3).reshape(128, -1)),
        "wd": bf(chunks(Wd3).reshape(128, -1)),
        "ba": bf(b2[:N1].reshape(128, 128) * (ALPHA * BETA)),
        "bb": bf(b2[N1:N1 + N2].reshape(128, 64) * (ALPHA * BETA)),
        "bc": bf(b2[N1 + N2:N1 + N2 + N3].reshape(64, 64) * BETA),
        "bd": bf(b2[N1 + N2 + N3:].reshape(64, 128) * BETA),
        "bcol": np.ascontiguousarray(bias.reshape(128, 1), f32),
        "ident": np.eye(128, dtype=f32),
    }

    in_maps = []
    for c in range(N_CORES):
        e0 = c * E
        m = dict(shared)
        m["wT"] = np.ascontiguousarray(wT[:, e0:e0 + E])
        m["finA"] = np.ascontiguousarray(
            finA[:, :, e0:e0 + E]).reshape(128, 8 * E)
        m["finCD"] = np.ascontiguousarray(
            finCD[:, :, e0:e0 + E]).reshape(128, 16 * E)
        m["s1t"] = bf(s1t[:, e0:e0 + E])
        m["vsall"] = bf(np.stack([v[:, e0:e0 + E] for v in vs], axis=1)
                        ).reshape(64, 3 * E)
        m["d2t"] = bf(dot12[:, e0:e0 + E])
        m["s2b"] = np.ascontiguousarray(s2b[:, e0:e0 + E])
        for i in range(3):
            m[f"v2b{i}"] = np.ascontiguousarray(v2b[i][:, e0:e0 + E])
        in_maps.append(m)
    return in_maps


def run(in_maps, **kwargs):
    nc = _get_nc()
    return run_bass_kernel_spmd(nc, in_maps, list(range(N_CORES)), **kwargs)


def kernel(data_in1, data_in2, weight, W0, b0, W1, b1, W2, b2, bias):
    in_maps = _prep_inputs(
        data_in1, data_in2, weight, W0, b0, W1, b1, W2, b2, bias
    )
    res = run(in_maps)
    out = np.concatenate(
        [np.asarray(res.results[c]["outp"]) for c in range(N_CORES)], axis=0
    )
    return out.astype(np.float32)


# revision 6
# speedup vs baseline: 1.9151x; 1.0933x over previous
"""Trainium2 Bass kernel for nn_O3TensorProductWeighted.

Computes, for each sample e:
    h  = relu(relu(weight @ W0 + b0) @ W1 + b1)           # [64]
    w  = h @ W2 + b2                                      # [36864] (never materialized)
    out0 = PW0*(einsum(Wa,s1)*s2 + I3*einsum(Wd,dot12))
    out1 = PW1*I3*(einsum(Wb,s1) x v2 + einsum(Wc,v1)*s2)
    out  = concat(out0, out1)/SQRT_K ; out[:128] += bias

Strategy: reassociate each einsum against the (k,u)-joint contraction of the
per-sample Khatri-Rao product h (x) x, so everything becomes dense matmuls
over shared W2 chunk weights, with the per-sample products built on-chip.
8 chunks of 8 k-values each; one PE broadcast matmul per chunk replicates
the 8 h2 rows 16x, ACT applies the relu (with a 2^2 scale folded in), and
tensor_mul ops (Pool for fp8 A/B products, DVE 2x mode for bf16 C/D) build
the per-sample Khatri-Rao products.

The A (0e x 0e) and B (0e x 1o) paths run as fp8-e4m3 DoubleRow matmuls
(two 128-row tiles summed per instruction at 0.5 cycles/row): weights are
split into an fp8 main + same-scale fp8 residual (two DR instructions) so
only the fp8 product rounding contributes error. C/D paths stay bf16.

Scheduling: C/D matmuls trail their chunk by 2 so the PE never waits on
the big finCD stream at the head; fp8 residual DR work is deferred to the
second half of the loop; the b2-bias matmuls ride mid-loop; small DRAM
parameters are packed into combined tensors to amortize per-DMA setup;
weight streams are chunk-sliced and interleaved. Scale compensation
(alpha*beta) is folded into host-prescaled epilogue operands. The epilogue
runs in two sample-halves so DVE work overlaps the PE transposes and
output DMA. Pure data parallel over 8 cores, transposed layout (features
on partitions, samples on the free dim).
"""

import dataclasses
import sys

sys.path.insert(0, "/opt/trn_rl_repo")

from contextlib import ExitStack

import ml_dtypes
import numpy as np

import concourse.bacc as bacc
import concourse.bass as bass
import concourse.tile as tile
from concourse import mybir
from concourse.bass_utils import run_bass_kernel_spmd

BF16 = mybir.dt.bfloat16
FP8 = mybir.dt.float8e4
F32 = mybir.dt.float32
BF16_NP = ml_dtypes.bfloat16
FP8_NP = ml_dtypes.float8_e4m3

N_CORES = 8
N = 4096
E = N // N_CORES  # 512 samples per core

MUL0, MUL1 = 128, 64
N1 = MUL0 * MUL0          # 16384
N2 = MUL0 * MUL1          # 8192
N3 = MUL1 * MUL1          # 4096
I3 = float(1.0 / np.sqrt(3.0))
# PW0/SQRT_K == 1.0 and PW1*I3/SQRT_K == 1.0 exactly; only I3 remains on D.

KPC = 8                   # k-values per chunk
G = 64 // KPC             # 8 chunks
TPB = 128 // KPC          # 16 u-values per row-block
NSL = 4                   # chunk-slices per big stationary stream
NJA = 4                   # DoubleRow pairs per chunk on A/B
NJC = 4                   # row-blocks per chunk on C/D
CD_LAG = 2                # C/D matmuls trail their chunk by this many slots
ALPHA = 1024.0            # fp8 scale on A/B stationary weights
BETA = 4.0                # scale folded into the relu -> rides on products
AB_INV = 1.0 / (ALPHA * BETA)
POOL_CD = {3}             # chunks whose C/D products build on Pool (A/B: all)


def _build_nc():
    nc = bacc.Bacc(None)

    dp = nc.declare_dram_parameter
    # packed startup params
    wTw0_d = dp("wTw0", [16, E + 64], BF16, isOutput=False)     # wT | w0
    b0c_d = dp("b0c", [64, 1], F32, isOutput=False)
    wg1_d = dp("wg1", [64, G * 128], BF16, isOutput=False)
    m128_d = dp("m128", [128, G + 1 + 128], F32, isOutput=False)  # bg1|bcol|ident
    # bigA: finA (8 blocks: s1 16-slices tiled 8x) | s1t | ba | bb
    bigA_d = dp("bigA", [128, 8 * E + E + 128 + 64], BF16, isOutput=False)
    # finCD halves: [C0,C1] and [C2,D] blocks (4 each of 16-slices tiled 8x)
    finCD_d = [dp(f"finCD{h}", [128, 8 * E], BF16, isOutput=False)
               for h in range(2)]
    # m64: vsall (3E) | d2t (E) | bc (64) | bd (128)
    m64_d = dp("m64", [64, 3 * E + E + 64 + 128], BF16, isOutput=False)
    v2p_d = dp("v2p", [64, 3 * E], F32, isOutput=False)
    s2b_d = dp("s2b", [128, E], F32, isOutput=False)
    # A/B stationaries: [128, G, NJA(jp), 2(tile), W] fp8 main + residual
    wa8m_d = dp("wa8m", [128, G * NJA * 2 * 128], FP8, isOutput=False)
    wa8r_d = dp("wa8r", [128, G * NJA * 2 * 128], FP8, isOutput=False)
    wb8m_d = dp("wb8m", [128, G * NJA * 2 * 64], FP8, isOutput=False)
    wb8r_d = dp("wb8r", [128, G * NJA * 2 * 64], FP8, isOutput=False)
    # C/D stationaries: [128, G, NJC(j), W] bf16
    wc_d = dp("wc", [128, G * NJC * 64], BF16, isOutput=False)
    wd_d = dp("wd", [128, G * NJC * 128], BF16, isOutput=False)

    outp_d = dp("outp", [E, 320], F32, isOutput=True)

    with tile.TileContext(nc) as tc, ExitStack() as ctx:
        const = ctx.enter_context(tc.tile_pool(name="const", bufs=1))
        work = ctx.enter_context(tc.tile_pool(name="work", bufs=1))
        bct_pool = ctx.enter_context(tc.tile_pool(name="bct", bufs=3))
        # pt8 tiles must survive ~4 chunks (residual DRs are deferred)
        pt8_pool = ctx.enter_context(tc.tile_pool(name="pt8", bufs=10))
        ptc_pool = ctx.enter_context(tc.tile_pool(name="ptc", bufs=4 + CD_LAG))
        out_pool = ctx.enter_context(tc.tile_pool(name="outs", bufs=2))
        ps_acc = ctx.enter_context(tc.tile_pool(name="ps_acc", bufs=1, space="PSUM"))
        ps_rot = ctx.enter_context(tc.tile_pool(name="ps_rot", bufs=2, space="PSUM"))

        def load(dparam, engine=None):
            t = const.tile(dparam.shape, dparam.dtype, name=f"t_{dparam.name}")
            (engine or nc.sync).dma_start(t[:], dparam[:])
            return t

        def sliced_tiles(dparam, n):
            w = dparam.shape[1] // n
            return [
                const.tile([dparam.shape[0], w], dparam.dtype,
                           name=f"t_{dparam.name}_{i}")
                for i in range(n)
            ], w

        # startup-critical on Pool SWDGE; streams on SP HWDGE by first use
        b0c_t = load(b0c_d, nc.gpsimd)
        wTw0_t = load(wTw0_d, nc.gpsimd)

        m128_t = load(m128_d)
        wg1_t = load(wg1_d)
        bigA_t = load(bigA_d)
        wa8m_ts, wa8m_w = sliced_tiles(wa8m_d, NSL)
        wb8m_ts, wb8m_w = sliced_tiles(wb8m_d, NSL)
        wc_ts, wc_w = sliced_tiles(wc_d, NSL)
        wd_ts, wd_w = sliced_tiles(wd_d, NSL)
        wa8r_ts, _ = sliced_tiles(wa8r_d, NSL)
        wb8r_ts, _ = sliced_tiles(wb8r_d, NSL)

        def stream(i):
            for ts, d, w in ((wa8m_ts, wa8m_d, wa8m_w), (wb8m_ts, wb8m_d, wb8m_w),
                             (wc_ts, wc_d, wc_w), (wd_ts, wd_d, wd_w),
                             (wa8r_ts, wa8r_d, wa8m_w), (wb8r_ts, wb8r_d, wb8m_w)):
                nc.sync.dma_start(ts[i][:], d[:, i * w: (i + 1) * w])

        nc.sync.dma_start(wa8m_ts[0][:], wa8m_d[:, 0:wa8m_w])
        nc.sync.dma_start(wb8m_ts[0][:], wb8m_d[:, 0:wb8m_w])
        finCD_t = [load(d) for d in finCD_d]
        nc.sync.dma_start(wc_ts[0][:], wc_d[:, 0:wc_w])
        nc.sync.dma_start(wd_ts[0][:], wd_d[:, 0:wd_w])
        nc.sync.dma_start(wa8r_ts[0][:], wa8r_d[:, 0:wa8m_w])
        nc.sync.dma_start(wb8r_ts[0][:], wb8r_d[:, 0:wb8m_w])
        for i in range(1, NSL):
            stream(i)
        m64_t = load(m64_d)
        v2p_t = load(v2p_d)
        s2b_t = load(s2b_d)

        # unpack views
        wT_v = wTw0_t[:, 0:E]
        w0_v = wTw0_t[:, E:E + 64]
        bg1_v = m128_t[:, 0:G]
        bcol_v = m128_t[:, G:G + 1]
        ident_v = m128_t[:, G + 1:]
        fA = bigA_t[:, 0:8 * E].rearrange("p (b e) -> p b e", b=8)
        s1t_v = bigA_t[:, 8 * E:9 * E]
        ba_v = bigA_t[:, 9 * E:9 * E + 128]
        bb_v = bigA_t[:, 9 * E + 128:9 * E + 192]
        fCD = [t[:].rearrange("p (b e) -> p b e", b=8) for t in finCD_t]
        vs3 = m64_t[:, 0:3 * E].rearrange("p (b e) -> p b e", b=3)
        d2t_v = m64_t[:, 3 * E:4 * E]
        bc_v = m64_t[:, 4 * E:4 * E + 64]
        bd_v = m64_t[:, 4 * E + 64:4 * E + 192]
        v2p = v2p_t[:].rearrange("p (b e) -> p b e", b=3)

        GS = G // NSL  # chunks per slice

        def wview(ts, g, jdim, tdim, w):
            return ts[g // GS][:].rearrange(
                "p (g j t w) -> p g j t w", g=GS, j=jdim, t=tdim)[:, g % GS]

        # MLP layer 1: h1 = relu(W0.T @ wT + b0) : [64, E]
        ps_h1 = ps_rot.tile([64, E], F32, tag="rot")
        nc.tensor.matmul(ps_h1[:], w0_v, wT_v, start=True, stop=True)
        h1_t = work.tile([64, E], BF16)
        nc.scalar.activation(
            h1_t[:], ps_h1[:], mybir.ActivationFunctionType.Relu,
            bias=b0c_t[:], scale=1.0,
        )

        # persistent PSUM accumulators
        psA = ps_acc.tile([128, E], F32, tag="A")
        psB = ps_acc.tile([64, E], F32, tag="B")
        psC = [ps_acc.tile([64, E], F32, tag=f"C{i}", name=f"psC{i}")
               for i in range(3)]
        psD = ps_acc.tile([128, E], F32, tag="D")

        def bcast(g):
            ps_bc = ps_rot.tile([128, E], F32, tag="rot", name=f"bc{g}")
            nc.tensor.matmul(ps_bc[:], wg1_t[:, bass.ts(g, 128)], h1_t[:],
                             start=True, stop=True, skip_group_check=True)
            return ps_bc

        DR = mybir.MatmulPerfMode.DoubleRow
        pt8s = [None] * G   # retained fp8 product tiles (deferred residuals)
        ptcs = [None] * G   # retained bf16 product tiles (lagged C/D)

        def emit_res(g, stop):
            p2 = pt8s[g][:].rearrange("p (b e) -> p b e", b=2 * NJA)
            wa = wview(wa8r_ts, g, NJA, 2, 128)
            wb = wview(wb8r_ts, g, NJA, 2, 64)
            for jp in range(NJA):
                nc.tensor.matmul(psA[:], wa[:, jp], p2[:, 2 * jp:2 * jp + 2, :],
                                 start=False, stop=stop and jp == NJA - 1,
                                 perf_mode=DR, skip_group_check=True)
            for jp in range(NJA):
                nc.tensor.matmul(psB[:], wb[:, jp], p2[:, 2 * jp:2 * jp + 2, :],
                                 start=False, stop=stop and jp == NJA - 1,
                                 perf_mode=DR, skip_group_check=True)

        def emit_cd(g, last):
            pc0 = ptcs[g][0][:].rearrange("p (b e) -> p b e", b=8)
            pc1 = ptcs[g][1][:].rearrange("p (b e) -> p b e", b=8)
            wcv = wview(wc_ts, g, NJC, 1, 64)
            wdv = wview(wd_ts, g, NJC, 1, 128)
            for i in range(3):
                pc = pc0 if i < 2 else pc1
                off = NJC * i if i < 2 else 0
                for j in range(NJC):
                    nc.tensor.matmul(psC[i][:], wcv[:, j], pc[:, off + j, :],
                                     start=(g == 0 and j == 0),
                                     stop=last and j == NJC - 1,
                                     skip_group_check=True)
            for j in range(NJC):
                nc.tensor.matmul(psD[:], wdv[:, j], pc1[:, NJC + j, :],
                                 start=(g == 0 and j == 0),
                                 stop=last and j == NJC - 1,
                                 skip_group_check=True)

        ps_bc = bcast(0)
        for g in range(G + CD_LAG):
            if g < G:
                bct = bct_pool.tile([128, E], BF16, tag="bct")
                nc.scalar.activation(
                    bct[:], ps_bc[:], mybir.ActivationFunctionType.Relu,
                    bias=bg1_v[:, g: g + 1], scale=BETA,
                )
                # products: pt8 (A/B on Pool, fp8) ; ptc halves (C/D, bf16)
                pt8 = pt8_pool.tile([128, 2 * NJA * E], FP8, tag="pt8")
                pt8s[g] = pt8
                bct_b = dataclasses.replace(
                    bct[:], ap=[bct[:].ap[0], [0, 2 * NJA], [1, E]]
                )
                nc.gpsimd.tensor_mul(
                    pt8[:].rearrange("p (b e) -> p b e", b=2 * NJA), fA, bct_b)
                cd_eng = nc.gpsimd if g in POOL_CD else nc.vector
                bct_c = dataclasses.replace(
                    bct[:], ap=[bct[:].ap[0], [0, 8], [1, E]]
                )
                ptcs[g] = []
                for h in range(2):
                    ptc = ptc_pool.tile([128, 8 * E], BF16, tag="ptc")
                    ptcs[g].append(ptc)
                    cd_eng.tensor_mul(
                        ptc[:].rearrange("p (b e) -> p b e", b=8),
                        fCD[h], bct_c)
                if g + 1 < G:
                    ps_bc = bcast(g + 1)

                p2 = pt8[:].rearrange("p (b e) -> p b e", b=2 * NJA)
                wa = wview(wa8m_ts, g, NJA, 2, 128)
                wb = wview(wb8m_ts, g, NJA, 2, 64)
                for jp in range(NJA):
                    nc.tensor.matmul(psA[:], wa[:, jp],
                                     p2[:, 2 * jp:2 * jp + 2, :],
                                     start=(g == 0 and jp == 0), stop=False,
                                     perf_mode=DR, skip_group_check=True)
                for jp in range(NJA):
                    nc.tensor.matmul(psB[:], wb[:, jp],
                                     p2[:, 2 * jp:2 * jp + 2, :],
                                     start=(g == 0 and jp == 0), stop=False,
                                     perf_mode=DR, skip_group_check=True)
            if g >= CD_LAG:
                emit_cd(g - CD_LAG, last=g == G + CD_LAG - 1)
            if g == G // 2:
                # b2 contributions ride mid-loop (their DMAs arrived by now)
                nc.tensor.matmul(psA[:], ba_v, s1t_v, start=False,
                                 stop=False, skip_group_check=True)
                nc.tensor.matmul(psB[:], bb_v, s1t_v, start=False,
                                 stop=False, skip_group_check=True)
                for i in range(3):
                    nc.tensor.matmul(psC[i][:], bc_v, vs3[:, i, :],
                                     start=False, stop=False,
                                     skip_group_check=True)
                nc.tensor.matmul(psD[:], bd_v, d2t_v, start=False,
                                 stop=False, skip_group_check=True)
            # deferred fp8 residual passes (weights arrive late; order-free)
            if G // 2 <= g < G:
                emit_res(2 * (g - G // 2), stop=False)
                emit_res(2 * (g - G // 2) + 1, stop=g == G - 1)

        # epilogue in two sample-halves so DVE overlaps PE transposes + DMA:
        # out0T = psA*s2/(ab) + (I3/b)*psD + bias ; out1T_i = psB*v2_i/(ab) + psC_i/b
        out0T = work.tile([128, E], F32)
        out1T = [work.tile([64, E], F32, tag=f"o1{i}", name=f"o1{i}")
                 for i in range(3)]
        for h in range(2):
            sl = bass.ts(h, E // 2)
            tA = work.tile([128, E // 2], F32, tag=f"tA{h}", name=f"tA{h}")
            nc.vector.tensor_mul(tA[:], psA[:, sl], s2b_t[:, sl])
            tD = work.tile([128, E // 2], F32, tag=f"tD{h}", name=f"tD{h}")
            nc.scalar.mul(tD[:], psD[:, sl], I3 / BETA)
            nc.vector.affine_then_add(out0T[:, sl], tA[:], tD[:], scale=1.0,
                                      bias=bcol_v)
            for i in range(3):
                tB = work.tile([64, E // 2], F32, tag=f"tB{i}{h}",
                               name=f"tB{i}{h}")
                nc.vector.tensor_mul(tB[:], psB[:, sl], v2p[:, i, sl])
                nc.vector.affine_then_add(out1T[i][:, sl], psC[i][:, sl],
                                          tB[:], scale=1.0 / BETA, bias=0.0)
            # transpose this half back to [E, 320] and store
            for eh in range(2):
                et = 2 * h + eh
                slt = bass.ts(et, 128)
                outS = out_pool.tile([128, 320], F32, tag="outS")
                ps_t0 = ps_rot.tile([128, E], F32, tag="rot")
                nc.tensor.transpose(ps_t0[:, 0:128], out0T[:, slt], ident_v)
                nc.scalar.copy(outS[:, 0:128], ps_t0[:, 0:128])
                o1v = outS[:, 128:320].rearrange("p (w i) -> p i w", i=3)
                for i in range(3):
                    ps_ti = ps_rot.tile([128, E], F32, tag="rot")
                    nc.tensor.transpose(ps_ti[:, 0:64], out1T[i][:, slt],
                                        ident_v[0:64, 0:64])
                    nc.scalar.copy(o1v[:, i, :], ps_ti[:, 0:64])
                nc.sync.dma_start(outp_d[slt, :], outS[:])

    nc.compile()
    return nc


_NC = None


def _get_nc():
    global _NC
    if _NC is None:
        _NC = _build_nc()
    return _NC


def _q8(x):
    return np.clip(x, -240.0, 240.0).astype(FP8_NP)


def _prep_inputs(data_in1, data_in2, weight, W0, b0, W1, b1, W2, b2, bias):
    f32 = np.float32
    data_in1 = np.ascontiguousarray(data_in1, dtype=f32)
    data_in2 = np.ascontiguousarray(data_in2, dtype=f32)
    weight = np.ascontiguousarray(weight, dtype=f32)
    W0 = np.asarray(W0, f32); b0 = np.asarray(b0, f32)
    W1 = np.asarray(W1, f32); b1 = np.asarray(b1, f32)
    W2 = np.asarray(W2, f32); b2 = np.asarray(b2, f32)
    bias = np.asarray(bias, f32)

    s1 = data_in1[:, :MUL0]                      # [N,128]
    v1 = data_in1[:, MUL0:].reshape(N, MUL1, 3)  # [N,64,3]
    s2 = data_in2[:, 0]                          # [N]
    v2 = data_in2[:, 1:4]                        # [N,3]

    def bf(x):
        return np.ascontiguousarray(x, dtype=f32).astype(BF16_NP)

    s1t = s1.T                                   # [128,N] f32
    dot12 = np.einsum("eui,ei->eu", v1, v2).T    # [64,N]
    vs = [(v1[:, :, i] * s2[:, None]).T for i in range(3)]  # [64,N] each

    def tiles_of(x, nblk):
        return [np.tile(x[TPB * j: TPB * (j + 1)], (KPC, 1))
                for j in range(nblk)]

    finA = np.stack(tiles_of(s1t, 8), axis=1)            # [128, 8, N]
    blk0 = tiles_of(vs[0], 4) + tiles_of(vs[1], 4)       # C0 | C1
    blk1 = tiles_of(vs[2], 4) + tiles_of(dot12, 4)       # C2 | D
    finCD0 = np.stack(blk0, axis=1)                      # [128, 8, N]
    finCD1 = np.stack(blk1, axis=1)

    wT = bf(weight.T)
    s2b = np.ascontiguousarray(
        np.broadcast_to(s2 * AB_INV, (128, N)), dtype=f32)

    # stationary chunk layouts: row r=(koff*TPB+uu) -> W[KPC*g+koff, TPB*j+uu, :]
    def chunks(arr3):  # [64,U,W] -> [128(r), G, U//TPB(j), W]
        U, W = arr3.shape[1], arr3.shape[2]
        t = arr3.reshape(G, KPC, U // TPB, TPB, W)     # [g,koff,j,uu,w]
        return np.transpose(t, (1, 3, 0, 2, 4)).reshape(128, G, U // TPB, W)

    Wa3 = W2[:, :N1].reshape(64, 128, 128)
    Wb3 = W2[:, N1:N1 + N2].reshape(64, 128, 64)
    Wc3 = W2[:, N1 + N2:N1 + N2 + N3].reshape(64, 64, 64)
    Wd3 = W2[:, N1 + N2 + N3:].reshape(64, 64, 128)

    # fp8 main + same-scale residual for A/B
    def fp8_pair(arr3):
        c = chunks(arr3) * ALPHA                       # [128, G, U//TPB, W]
        m = _q8(c)
        r = _q8(c - m.astype(f32))
        return (np.ascontiguousarray(m).reshape(128, -1),
                np.ascontiguousarray(r).reshape(128, -1))

    wa8m, wa8r = fp8_pair(Wa3)
    wb8m, wb8r = fp8_pair(Wb3)

    bg1 = (b1.reshape(G, KPC)[None, :, :].repeat(TPB, axis=0)
           .transpose(2, 0, 1).reshape(128, G) * BETA)
    m128 = np.concatenate(
        [bg1, bias.reshape(128, 1), np.eye(128, dtype=f32)], axis=1
    ).astype(f32)
    m64_shared = [bf(b2[N1 + N2:N1 + N2 + N3].reshape(64, 64) * BETA),
                  bf(b2[N1 + N2 + N3:].reshape(64, 128) * BETA)]
    ba = bf(b2[:N1].reshape(128, 128) * (ALPHA * BETA))
    bb = bf(b2[N1:N1 + N2].reshape(128, 64) * (ALPHA * BETA))

    shared = {
        "b0c": np.ascontiguousarray(b0.reshape(64, 1), f32),
        "wg1": bf(np.repeat(W1, TPB, axis=1)),
        "m128": np.ascontiguousarray(m128),
        "wa8m": wa8m, "wa8r": wa8r, "wb8m": wb8m, "wb8r": wb8r,
        "wc": bf(chunks(Wc3).reshape(128, -1)),
        "wd": bf(chunks(Wd3).reshape(128, -1)),
    }
    w0b = bf(W0)

    in_maps = []
    for c in range(N_CORES):
        e0 = c * E
        sl = slice(e0, e0 + E)
        m = dict(shared)
        m["wTw0"] = np.ascontiguousarray(
            np.concatenate([wT[:, sl], w0b], axis=1))
        m["bigA"] = np.ascontiguousarray(np.concatenate(
            [bf(finA[:, :, sl]).reshape(128, 8 * E), bf(s1t[:, sl]), ba, bb],
            axis=1))
        m["finCD0"] = bf(finCD0[:, :, sl]).reshape(128, 8 * E)
        m["finCD1"] = bf(finCD1[:, :, sl]).reshape(128, 8 * E)
        m["m64"] = np.ascontiguousarray(np.concatenate(
            [bf(np.stack([v[:, sl] for v in vs], axis=1)).reshape(64, 3 * E),
             bf(dot12[:, sl])] + m64_shared, axis=1))
        m["v2p"] = np.ascontiguousarray(np.stack(
            [np.broadcast_to(v2[:, i] * AB_INV, (64, N))[:, sl]
             for i in range(3)], axis=1).reshape(64, 3 * E), f32)
        m["s2b"] = np.ascontiguousarray(s2b[:, sl])
        in_maps.append(m)
    return in_maps


def run(in_maps, **kwargs):
    nc = _get_nc()
    return run_bass_kernel_spmd(nc, in_maps, list(range(N_CORES)), **kwargs)


def kernel(data_in1, data_in2, weight, W0, b0, W1, b1, W2, b2, bias):
    in_maps = _prep_inputs(
        data_in1, data_in2, weight, W0, b0, W1, b1, W2, b2, bias
    )
    res = run(in_maps)
    out = np.concatenate(
        [np.asarray(res.results[c]["outp"]) for c in range(N_CORES)], axis=0
    )
    return out.astype(np.float32)
